# revision 63
# baseline (speedup 1.0000x reference)
"""Trainium2 Bass kernel for nn_ConvSPE (depthwise-conv SPE + per-channel contraction).

Math (reference): per bn=(b,nu) row and channel d:
    pe_k = noise / sqrt(num*d)                       (b*num, d, s+2k)
    pe_q = depthwise_valid_xcorr(pe_k, w)            k=200 taps, same filter per channel
    qhat[b,nu,t] = sum_d pe_q[bn,d,t]      * q[b,d,t]
    khat[b,nu,t] = sum_d pe_k[bn,d,t+k//2] * k[b,d,t]

Kernel strategy (8 NeuronCores, data-parallel over the 128 bn rows; 16 rows/core):
  * Host pre-arranges noise into a time-partition-inner fp16 layout
    xf[p, row, n, d] = noise[row, d, 128n+p] so the conv becomes 3 PSUM-accumulated
    TensorE matmuls per output block with fixed Toeplitz weights
    W_s[p, m] = w[p + 128s - m] (shared across all channels/rows).
  * qhat: ScalarE (ACT) drains conv PSUM -> SBUF fp16; DVE multiplies in-place by
    host-pre-transposed queries (fp16 2x mode); d-reduction via fp16 binary
    add-trees batched over row pairs.
  * khat needs no conv: DVE multiplies xf by a host-shifted/scaled keys layout;
    d-reduction via the same pairwise add-tree.
  * Tree L1 levels (the big ones) go to GpSimd per a tunable split table to
    balance DVE vs GpSimd busy time; the final pair stays on DVE to shorten the
    pipeline tail. Emission is software-pipelined: pair p's trees are emitted
    after row 2p+2's muls so tree work overlaps the next rows' conv.
  * Outputs stay fp16 on device; host casts to fp32.
"""

import math
import numpy as np

_CACHE = {}


def _ensure_paths():
    try:
        import concourse  # noqa: F401
    except ImportError:
        import sys

        for p in ("/opt/trn_rl_repo", "/root/.axon_site/_ro/trn_rl_repo"):
            if p not in sys.path:
                sys.path.insert(0, p)


N_CORES = 8
B, D, L, K, NUM = 4, 64, 4096, 200, 32
NW = 34  # x windows of 128 loaded per row (covers t+j up to 4351)
NT = 32  # output time blocks of 128
NK = 33  # khat product blocks (u = t + 100 spans [0, 4224))
ROWS = 16  # bn rows per core
NPAIR = ROWS // 2

# Tree levels on GpSimd ("pool") vs DVE, per (path, pair). Tunable. (The Q7
# only implements TensorTensor adds; TensorScalarPtr is rejected by codegen.)
POOL_Q_L1 = {0, 1, 2, 3, 4, 5, 6}
POOL_L2 = {0, 1, 2, 3, 4, 5}  # pairs whose qhat L2 level runs on GpSimd
# rows whose khat L1 is pre-reduced before the PE accumulation (halves those
# rows' PE columns): early rows on GpSimd (fills its idle fill window), middle
# rows on DVE; the last rows stay PE-only so the DVE/GpSimd tails stay light.
KL1_POOL_ROWS = set()
KL1_DVE_ROWS = set(range(10))


def build_module():
    """Build + compile the per-core Bass module (identical SPMD program)."""
    _ensure_paths()
    from contextlib import ExitStack

    import concourse.bacc as bacc
    import concourse.mybir as mybir
    import concourse.tile as tile

    F16 = mybir.dt.float16
    F32 = mybir.dt.float32
    ADD = mybir.AluOpType.add
    BYP = mybir.AluOpType.bypass

    nc = bacc.Bacc(
        "TRN2", target_bir_lowering=False, debug=False, num_devices=N_CORES
    )

    xf_d = nc.dram_tensor("xf", [128, ROWS, NW, D], F16, kind="ExternalInput").ap()
    wq_d = nc.dram_tensor("wq", [4, 128, 128], F16, kind="ExternalInput").ap()
    qt_d = nc.dram_tensor("qt", [128, NT, D], F16, kind="ExternalInput").ap()
    kf_d = nc.dram_tensor("kf", [128, NK, D], F16, kind="ExternalInput").ap()
    qo_d = nc.dram_tensor("qo", [128, ROWS, NT], F16, kind="ExternalOutput").ap()
    ko_d = nc.dram_tensor("ko", [128, ROWS, NK], F16, kind="ExternalOutput").ap()

    with tile.TileContext(nc) as tc, ExitStack() as ctx:
        wp = ctx.enter_context(tc.tile_pool(name="const", bufs=1))
        xp = ctx.enter_context(tc.tile_pool(name="x", bufs=5))
        pp = ctx.enter_context(tc.tile_pool(name="psum", bufs=3, space="PSUM"))
        pk_ps = ctx.enter_context(tc.tile_pool(name="psumK", bufs=2, space="PSUM"))
        cp = ctx.enter_context(tc.tile_pool(name="peq", bufs=5))
        kp = ctx.enter_context(tc.tile_pool(name="pk", bufs=5))
        tk = ctx.enter_context(tc.tile_pool(name="treeK", bufs=3))
        tq = ctx.enter_context(tc.tile_pool(name="treeQ", bufs=3))
        op = ctx.enter_context(tc.tile_pool(name="out", bufs=1))

        # one DMA for all three Toeplitz weights (saves two DGE issue slots
        # ahead of the first xt transfer)
        wall = wp.tile([128, 4, 128], F16, tag="wall")
        nc.sync.dma_start(wall[:], wq_d.transpose([1, 0, 2]))
        wts = [wall[:, s, :] for s in range(3)]
        ident = wall[:, 3, :]
        kf_t = wp.tile([128, NK, D], F16, tag="kf")
        qt_t = wp.tile([128, NT, D], F16, tag="qt")

        qacc = op.tile([128, ROWS, NT], F16, tag="qa")
        kacc = op.tile([128, ROWS, NK], F16, tag="ka")

        xts, pks, peqs, trees = {}, {}, {}, {}

        def pool_add(out, in0, in1):
            nc.gpsimd.tensor_add(out, in0, in1)

        def emit_row(r):
            p = r // 2
            xt = xp.tile([128, NW, D], F16, tag="xt", name=f"xt_{r}")
            if r == 0:
                # row 0's xt + qt transfer on the ACT DGE queue, in parallel
                # with kf on the SP queue, so the first kmul starts earlier
                nc.sync.dma_start(kf_t[:], kf_d[:])
                nc.scalar.dma_start(xt[:], xf_d[:, r])
                nc.scalar.dma_start(qt_t[:], qt_d[:])
            else:
                nc.sync.dma_start(xt[:], xf_d[:, r])
            if r % 2 == 0:
                aQ = tq.tile([128, 2, NT, 32], F16, tag="aQ", name=f"aQ_{p}")
                bQ = tq.tile([128, 2, NT, 16], F16, tag="bQ", name=f"bQ_{p}")
                trees[p] = (aQ, bQ)

            # khat product (DVE, fp16 2x); its d-reduction happens on PE below,
            # optionally after one DVE pre-reduction level
            pk = kp.tile([128, NK, D], F16, tag="pk", name=f"pk_{r}")
            nc.vector.tensor_mul(pk[:], xt[:, 0:NK, :], kf_t[:])
            pks[r] = pk
            if r in KL1_DVE_ROWS or r in KL1_POOL_ROWS:
                pl = kp.tile([128, NK, 32], F16, tag="pkl", name=f"pkl_{r}")
                if r in KL1_POOL_ROWS:
                    pool_add(pl[:], pk[:, :, 0:32], pk[:, :, 32:64])
                else:
                    nc.vector.tensor_add(pl[:], pk[:, :, 0:32], pk[:, :, 32:64])
                pks[r] = pl

            # conv + drains + qhat product + its L1
            peq = cp.tile([128, NT, D], F16, tag="peq", name=f"peq_{r}")
            for h in range(2):
                ps = pp.tile([128, NT // 2, D], F32, tag="ps", name=f"ps_{r}_{h}")
                for s in range(3):
                    for g in range(2 * h, 2 * h + 2):
                        nc.tensor.matmul(
                            ps[:, (g - 2 * h) * 8 : (g - 2 * h + 1) * 8, :],
                            wts[s],
                            xt[:, g * 8 + s : g * 8 + s + 8, :],
                            start=(s == 0),
                            stop=(s == 2),
                        )
                sl = slice(h * (NT // 2), (h + 1) * (NT // 2))
                nc.scalar.copy(peq[:, sl, :], ps[:])
            peqs[r] = peq

            # khat d-reduction for the PREVIOUS row (one row of lag so the
            # L1 pre-reductions are ready before PE consumes them)
            if r >= 1:
                emit_khat_pe(r - 1)

        def emit_khat_pe(r):
            # identity matmuls accumulate pks[r][:, :, d] into one PSUM region
            # (free size 33 per matmul); ACT drains the [128, NK] result
            src_t = pks[r]
            width = src_t[:].shape[2]
            psK = pk_ps.tile([128, NK], F32, tag="psK", name=f"psK_{r}")
            for d in range(width):
                nc.tensor.matmul(psK[:], ident, src_t[:, :, d],
                                 start=(d == 0), stop=(d == width - 1))
            nc.scalar.copy(kacc[:, r, :], psK[:])

        def emit_qmul(r):
            p = r // 2
            peq = peqs[r]
            nc.vector.tensor_mul(peq[:], peq[:], qt_t[:])
            aQ = trees[p][0]
            if p in POOL_Q_L1:
                pool_add(aQ[:, r % 2], peq[:, :, 0:32], peq[:, :, 32:64])
            else:
                nc.vector.tensor_add(aQ[:, r % 2], peq[:, :, 0:32], peq[:, :, 32:64])

        def emit_pair_rest(p):
            r0 = 2 * p
            aQ, bQ = trees[p]
            if p in POOL_L2:
                pool_add(bQ[:], aQ[:, :, :, 0:16], aQ[:, :, :, 16:32])
            else:
                nc.vector.tensor_add(bQ[:], aQ[:, :, :, 0:16], aQ[:, :, :, 16:32])
            nc.vector.tensor_add(aQ[:, :, :, 0:8], bQ[:, :, :, 0:8], bQ[:, :, :, 8:16])
            nc.vector.tensor_add(bQ[:, :, :, 0:4], aQ[:, :, :, 0:4], aQ[:, :, :, 4:8])
            nc.vector.tensor_add(aQ[:, :, :, 8:10], bQ[:, :, :, 0:2], bQ[:, :, :, 2:4])
            nc.vector.tensor_add(
                qacc[:, r0 : r0 + 2, :], aQ[:, :, :, 8], aQ[:, :, :, 9]
            )

        # qmul lags one row and tree tails lag two pairs behind production, so
        # DVE never idles waiting for the drains / GpSimd L1s
        for r in range(ROWS):
            emit_row(r)
            if r >= 1:
                emit_qmul(r - 1)
            if r % 2 == 0 and r >= 6:
                emit_pair_rest(r // 2 - 3)
        emit_khat_pe(ROWS - 1)
        emit_qmul(ROWS - 1)
        emit_pair_rest(NPAIR - 3)
        # pairs 0-5 are fully reduced: ship them while the tail computes
        nc.sync.dma_start(qo_d[:, 0 : 2 * (NPAIR - 2)], qacc[:, 0 : 2 * (NPAIR - 2), :])
        nc.sync.dma_start(ko_d[:, 0 : 2 * (NPAIR - 2)], kacc[:, 0 : 2 * (NPAIR - 2), :])
        # last pair (DVE-only L1s) first: it is ready before the GpSimd queue
        # drains, so DVE finishes it while pair 6's L1s are still on GpSimd
        emit_pair_rest(NPAIR - 1)
        emit_pair_rest(NPAIR - 2)
        nc.sync.dma_start(qo_d[:, 2 * (NPAIR - 2) :], qacc[:, 2 * (NPAIR - 2) :, :])
        nc.sync.dma_start(ko_d[:, 2 * (NPAIR - 2) :], kacc[:, 2 * (NPAIR - 2) :, :])

    nc.compile()
    return nc


def _get_module():
    if "nc" not in _CACHE:
        _CACHE["nc"] = build_module()
    return _CACHE["nc"]


def make_in_maps(queries, keys, noise, conv_weight, num):
    """Host-side shard + re-layout (all cheap numpy ops)."""
    num = int(np.asarray(num))
    queries = np.asarray(queries, dtype=np.float32)
    keys = np.asarray(keys, dtype=np.float32)
    noise = np.asarray(noise, dtype=np.float32)
    w = np.asarray(conv_weight, dtype=np.float32)[0, 0, :]
    scale = 1.0 / math.sqrt(num * D)

    # Toeplitz weights (scale folded in): W_s[p, m] = w[p + 128s - m] * scale
    p = np.arange(128)[:, None]
    m = np.arange(128)[None, :]
    Wq = np.zeros((3, 128, 128), np.float32)
    for s in range(3):
        j = p + 128 * s - m
        mask = (j >= 0) & (j < K)
        Wq[s][mask] = w[j[mask]] * scale
    Wq16 = np.concatenate(
        [Wq.astype(np.float16), np.eye(128, dtype=np.float16)[None]], axis=0
    )

    # xf[core][p, row, n, d] = noise[16c+row, d, 128n + p]
    xf = (
        noise[:, :, : NW * 128]
        .reshape(B * NUM, D, NW, 128)
        .transpose(3, 0, 2, 1)
        .astype(np.float16)
    )  # [128, B*NUM, NW, D]
    # qt[b][p, tau, d] = queries[b, d, 128 tau + p]
    qt = queries.reshape(B, D, NT, 128).transpose(0, 3, 2, 1).astype(np.float16)
    # kf[b][p, n, d] = keys[b, d, 128n + p - 100] * scale (zero out of range)
    kp = np.zeros((B, D, NK * 128), np.float32)
    kp[:, :, K // 2 : K // 2 + L] = keys * scale
    kf = kp.reshape(B, D, NK, 128).transpose(0, 3, 2, 1).astype(np.float16)

    in_maps = []
    for c in range(N_CORES):
        b = c // 2
        in_maps.append(
            {
                "xf": np.ascontiguousarray(xf[:, ROWS * c : ROWS * (c + 1)]),
                "wq": Wq16,
                "qt": np.ascontiguousarray(qt[b]),
                "kf": np.ascontiguousarray(kf[b]),
            }
        )
    return in_maps


def assemble_outputs(results):
    qhat = np.empty((B * NUM, L), np.float32)
    khat = np.empty((B * NUM, L), np.float32)
    for c in range(N_CORES):
        qo = results[c]["qo"]  # [128, ROWS, NT] fp16
        ko = results[c]["ko"]  # [128, ROWS, NK] fp16
        qhat[ROWS * c : ROWS * (c + 1)] = (
            qo.astype(np.float32).transpose(1, 2, 0).reshape(ROWS, L)
        )
        kv = ko.astype(np.float32).transpose(1, 2, 0).reshape(ROWS, NK * 128)
        khat[ROWS * c : ROWS * (c + 1)] = kv[:, K // 2 : K // 2 + L]
    return (
        qhat.reshape(B, NUM, L),
        khat.reshape(B, NUM, L),
    )


def kernel(queries, keys, noise, conv_weight, num):
    _ensure_paths()
    from concourse import bass_utils

    in_maps = make_in_maps(queries, keys, noise, conv_weight, num)
    nc = _get_module()
    res = bass_utils.run_bass_kernel_spmd(nc, in_maps, core_ids=list(range(N_CORES)))
    return assemble_outputs(res.results)
